# revision 22
# baseline (speedup 1.0000x reference)
"""Causal multi-head attention on 8 Trainium2 NeuronCores.

Problem: nn_Attention_46643344835180
  x: [8, 1024, 768], 12 heads x 64 dh, causal softmax attention + output proj.

Sharding: data-parallel over batch (8 batch elements -> 8 cores, no collectives).

Key optimizations vs the first working version (226-265us -> ~172-179us):
  - x transposed on the HOST (input is xT, bf16): removes 48 PE
    transposes + 48 DVE casts, shortens startup.
  - All matmul inputs bf16 (halves input DMA to 6.3MB; PE rate is
    1 col/cycle either way, PSUM accumulation stays fp32). rel err 3e-3
    vs the 2e-2 gate.
  - PE kept dense so the HAM p-state never drops to 1.2GHz: attn units
    are software-pipelined (scores of group g+1 issue before PV of g)
    and independent projection chains (qk/v/o) are threaded through as
    filler closures; o_proj for the first query half is spread across
    the c=1 attention sweep.
  - DMA issue cost (~700ns/issue, serial per sequencer) amortized:
    host-packed contiguous wq/wk slabs, batched loads, issues split
    across the sync AND Activation DGE queues, transfers split into
    halves across DMA queues.
  - exp skips leading fully-masked columns; ragged f32r score matmuls
    are widened to >=256 cols when not bf16 (4 cyc/col penalty below).

Per-core dataflow (batch element b):
  xT  (DMA, host-transposed, bf16)                                    [768, 1024]
  QT = Wq_cat.T @ xT  (+bq)   heads stacked on partitions             [768, 1024]
  KT = Wk_cat.T @ xT  (+bk)                                           [768, 1024]
  V  = xT.T @ Wv_cat  (+bv)   + interleaved ones column               [1024, 12*65]
  per head h, query-chunk qc (512):
    S^T[k,q] = KT_h.T @ QT_h          keys on partitions
    P^T = exp(S^T / 8)                ScalarE, batched over 2 key-blocks
    causal: one wide-mask multiply on the partial columns
    z^T[65,512] += [V_h | 1].T @ P^T  row 64 accumulates the denominator
    ZT_h = z^T[0:64] * recip(z^T[64]) (copy -> DVE recip -> gpsimd bcast -> mul)
  out = ZT.T @ Wo_cat (+bo)                                           [1024, 768]
"""

import sys

sys.path.insert(0, "/opt/trn_rl_repo")

import ml_dtypes
import numpy as np

import concourse.bass as bass
import concourse.mybir as mybir
import concourse.tile as tile
from concourse import bacc
from concourse.bass_utils import run_bass_kernel_spmd

F32 = mybir.dt.float32
F32R = mybir.dt.float32r
BF16 = mybir.dt.bfloat16
AF = mybir.ActivationFunctionType

SEQ = 1024
DM = 768
NH = 12
DH = 64
BATCH = 8
NQT = SEQ // 128  # 8 seq tiles of 128
NDT = DM // 128  # 6 d_model tiles
QC = 512  # query chunk (moving dim)
NQC = SEQ // QC  # 2

# (bf_qk, bf_pv, bf_o): bf_qk also covers xT and wv (matmul operand dtypes
# must match: qk_proj streams xT against wq/wk, v_proj uses xT as lhsT
# against wv).
CFG = (True, True, True)


def _npdt(dt):
    return ml_dtypes.bfloat16 if dt == BF16 else np.float32


def build(with_bq, with_bk, with_bv, with_bo, cfg=CFG):
    bf_qk, bf_pv, bf_o = cfg
    DT_QK = BF16 if bf_qk else F32R  # xT, wq/wk/wv, QT/KT, scores matmul
    DT_PV = BF16 if bf_pv else F32R  # V storage, ones, P^T, PV matmul
    DT_O = BF16 if bf_o else F32R  # ZT, wo, output matmul
    DT_MASK = BF16 if bf_pv else F32

    nc = bacc.Bacc("TRN2", target_bir_lowering=False, debug=False)

    xT_d = nc.dram_tensor("xT", [DM, SEQ], DT_QK, kind="ExternalInput")
    wq = nc.dram_tensor("wq", [DM, DM], DT_QK, kind="ExternalInput")
    wk = nc.dram_tensor("wk", [DM, DM], DT_QK, kind="ExternalInput")
    wv = nc.dram_tensor("wv", [DM, DM], DT_QK, kind="ExternalInput")
    wo = nc.dram_tensor("wo", [DM, DM], DT_O, kind="ExternalInput")
    wmask = nc.dram_tensor("wmask", [128, 640], DT_MASK, kind="ExternalInput")
    onesc = nc.dram_tensor("onesc", [128, NH], DT_PV, kind="ExternalInput")
    bq = bk = bv = bo = None
    if with_bq:
        bq = nc.dram_tensor("bq", [128, NDT], F32, kind="ExternalInput")
    if with_bk:
        bk = nc.dram_tensor("bk", [128, NDT], F32, kind="ExternalInput")
    if with_bv:
        bv = nc.dram_tensor("bv", [1, DM], F32, kind="ExternalInput")
    if with_bo:
        bo = nc.dram_tensor("bo", [1, DM], F32, kind="ExternalInput")
    out = nc.dram_tensor("out", [SEQ, DM], F32, kind="ExternalOutput")

    with tile.TileContext(nc) as tc:
        with (
            tc.tile_pool(name="persist", bufs=1) as persist,
            tc.tile_pool(name="wstream", bufs=6) as w_pool,
            tc.tile_pool(name="wqk", bufs=14) as wqk_pool,
            tc.tile_pool(name="pt", bufs=6) as pt_pool,
            tc.tile_pool(name="small", bufs=3) as small,
            tc.tile_pool(name="outst", bufs=2) as out_pool,
            tc.tile_pool(name="ps_st", bufs=2, space="PSUM") as ps_st,
            tc.tile_pool(name="ps_z", bufs=2, space="PSUM") as ps_z,
            tc.tile_pool(name="ps_mm", bufs=2, space="PSUM") as ps_mm,
        ):
            # ---- constants + first-needed weights ----
            wm_t = persist.tile([128, 640], DT_MASK, tag="wmask", name="wmask")
            nc.sync.dma_start(out=wm_t, in_=wmask[:, :])

            def qk_load(hp, pieces=1):
                # wq/wk are host-packed so each head-pair's weights are one
                # contiguous [128, 768] slab. pieces>1 splits the transfer
                # across DMA queues (each queue moves only ~30-50GB/s; issue
                # costs ~700ns on the sequencer) for startup-critical loads.
                tiles = []
                for wsrc in (wq, wk):
                    t = wqk_pool.tile([128, DM], DT_QK, tag="wqk", name="wqk")
                    w = DM // pieces
                    for i in range(pieces):
                        nc.sync.dma_start(
                            out=t[:, i * w : (i + 1) * w],
                            in_=wsrc[hp * 128 : (hp + 1) * 128, i * w : (i + 1) * w],
                        )
                    tiles.append([t[:, d * 128 : (d + 1) * 128] for d in range(NDT)])
                return tiles

            # ---- input loads, split across BOTH DMA-issue queues (each
            # issue costs ~700ns serial on its sequencer): sync takes the
            # first-needed halves, the Activation queue the rest in parallel.
            xT = [
                persist.tile([128, SEQ], DT_QK, tag=f"xT{d}", name=f"xT{d}")
                for d in range(NDT)
            ]
            NVC = 2
            VC = DM // NVC  # 384
            wt = [
                w_pool.tile([128, DM], DT_QK, tag="w", name="w")
                for d in range(NDT)
            ]
            qk_loads = {0: qk_load(0, pieces=2)}
            # xT halves: c=0 halves first (what qk_proj(0)/v_proj need),
            # split between both issue queues so transfers run in parallel
            for d in range(NDT):
                eng = nc.sync if d < 3 else nc.scalar
                eng.dma_start(
                    out=xT[d][:, 0:QC], in_=xT_d[d * 128 : (d + 1) * 128, 0:QC]
                )
            for d in range(NDT):
                eng = nc.sync if d < 3 else nc.scalar
                eng.dma_start(
                    out=xT[d][:, QC:SEQ],
                    in_=xT_d[d * 128 : (d + 1) * 128, QC:SEQ],
                )
            for d in range(NDT):
                eng = nc.sync if d < 3 else nc.scalar
                eng.dma_start(out=wt[d], in_=wv[d * 128 : (d + 1) * 128, :])

            # HAM warmup: dummy matmuls on a memset scratch tile (no DMA
            # dependency) while the input DMAs land, so the first projections
            # start warm instead of at the cold 0.65-1.2GHz
            warm_in = persist.tile([128, 128], DT_QK, tag="warm0", name="warm0")
            nc.vector.memset(warm_in, 0.0)
            warm_ps = ps_mm.tile(
                [128, 128], F32, tag="proj", name="warm", padded_shape=[128, QC]
            )
            for _ in range(24):
                nc.tensor.matmul(
                    warm_ps, lhsT=warm_in, rhs=warm_in, start=True, stop=True
                )

            qk_loads[1] = qk_load(1)

            bias_tiles = {}
            if with_bq:
                t = persist.tile([128, NDT], F32, tag="bq", name="bq")
                nc.sync.dma_start(out=t, in_=bq[:, :])
                bias_tiles["bq"] = t
            if with_bk:
                t = persist.tile([128, NDT], F32, tag="bk", name="bk")
                nc.sync.dma_start(out=t, in_=bk[:, :])
                bias_tiles["bk"] = t
            if with_bv:
                t = persist.tile([128, DM], F32, tag="bv", name="bv")
                nc.sync.dma_start(out=t, in_=bv[0:1, :].to_broadcast((128, DM)))
                bias_tiles["bv"] = t
            if with_bo:
                t = persist.tile([128, DM], F32, tag="bo", name="bo")
                nc.sync.dma_start(out=t, in_=bo[0:1, :].to_broadcast((128, DM)))
                bias_tiles["bo"] = t

            # ---- persistent activations ----
            QT = [
                persist.tile([128, SEQ], DT_QK, tag=f"QT{d}", name=f"QT{d}")
                for d in range(NDT)
            ]
            KT = [
                persist.tile([128, SEQ], DT_QK, tag=f"KT{d}", name=f"KT{d}")
                for d in range(NDT)
            ]
            V = [
                persist.tile([128, NH * (DH + 1)], DT_PV, tag=f"V{s}", name=f"V{s}")
                for s in range(NQT)
            ]
            for s in range(NQT):
                vv = V[s].rearrange("p (h e) -> p h e", e=DH + 1)
                nc.scalar.dma_start(
                    out=vv[:, :, DH : DH + 1],
                    in_=onesc[:, :].rearrange("p (h o) -> p h o", o=1),
                )
            ZT = [
                persist.tile([128, SEQ], DT_O, tag=f"ZT{d}", name=f"ZT{d}")
                for d in range(NDT)
            ]

            # ---- projection chain builders (each returns a PE-work closure) ----
            def qk_chain(wts, dst, hp, c, bkey):
                def run():
                    acc = ps_mm.tile([128, QC], F32, tag="proj", name="proj")
                    for d in range(NDT):
                        nc.tensor.matmul(
                            acc,
                            lhsT=wts[d],
                            rhs=xT[d][:, c * QC : (c + 1) * QC],
                            start=(d == 0),
                            stop=(d == NDT - 1),
                        )
                    o = dst[:, c * QC : (c + 1) * QC]
                    if bkey in bias_tiles:
                        nc.vector.tensor_scalar_add(
                            o, acc, bias_tiles[bkey][:, hp : hp + 1]
                        )
                    else:
                        nc.vector.tensor_copy(o, acc)

                return run

            def qk_proj_fillers(hp, tiles):
                fs = []
                for wts, (dst, bkey) in zip(tiles, ((QT, "bq"), (KT, "bk"))):
                    for c in range(NQC):
                        fs.append(qk_chain(wts, dst[hp], hp, c, bkey))
                return fs

            def v_chain(s, c, pool, tag):
                def run():
                    acc = pool.tile(
                        [128, VC], F32, tag=tag, name="vacc",
                        padded_shape=[128, 2 * QC] if tag == "st" else [128, QC],
                    )
                    for d in range(NDT):
                        nc.tensor.matmul(
                            acc,
                            lhsT=xT[d][:, s * 128 : (s + 1) * 128],
                            rhs=wt[d][:, c * VC : (c + 1) * VC],
                            start=(d == 0),
                            stop=(d == NDT - 1),
                        )
                    nh2 = VC // DH  # heads per chunk (6)
                    o = V[s].rearrange("p (h e) -> p h e", e=DH + 1)[
                        :, c * nh2 : (c + 1) * nh2, 0:DH
                    ]
                    if "bv" in bias_tiles:
                        nc.vector.tensor_add(
                            o,
                            acc.rearrange("p (h e) -> p h e", e=DH),
                            bias_tiles["bv"][:, c * VC : (c + 1) * VC].rearrange(
                                "p (h e) -> p h e", e=DH
                            ),
                        )
                    else:
                        nc.vector.tensor_copy(
                            o, acc.rearrange("p (h e) -> p h e", e=DH)
                        )

                return run

            wo_tiles = []

            def o_chain(s, c, ot, pool, tag):
                def run():
                    acc = pool.tile(
                        [128, VC], F32, tag=tag, name="oacc", padded_shape=[128, QC]
                    )
                    for d in range(NDT):
                        nc.tensor.matmul(
                            acc,
                            lhsT=ZT[d][:, s * 128 : (s + 1) * 128],
                            rhs=wo_tiles[d][:, c * VC : (c + 1) * VC],
                            start=(d == 0),
                            stop=(d == NDT - 1),
                        )
                    o = ot[:, c * VC : (c + 1) * VC]
                    if "bo" in bias_tiles:
                        nc.vector.tensor_add(
                            o, acc, bias_tiles["bo"][:, c * VC : (c + 1) * VC]
                        )
                    else:
                        nc.vector.tensor_copy(o, acc)
                    if c == NVC - 1:
                        nc.sync.dma_start(out=out[s * 128 : (s + 1) * 128, :], in_=ot)

                return run

            # ---- pipelined attention unit (head-pair hp, query chunk c) ----
            def attn_unit(hp, c, fillers=()):
                fillers = list(fillers)

                def filler():
                    # keep the last filler in reserve: it runs after the final
                    # PV group, covering the denominator-chain latency tail
                    if len(fillers) > 1:
                        fillers.pop(0)()

                zps = {}
                for px in (0, 64):  # head A in partitions 0:64, B in 64:128
                    zps[px] = ps_z.tile([128, QC], F32, tag="z", name="z")
                nkb = 4 * (c + 1)  # causal: key blocks 0..nkb-1
                groups = []
                for g in range(0, nkb, 2):
                    gsz = min(2, nkb - g)
                    # columns [0:doff) of a diagonal block are fully causal-
                    # masked: skip them in scores and PV (ragged-N). For f32r
                    # scores, widen to >=256 cols (narrow f32r matmuls run at
                    # 4 cycles/col at full clock); extra columns hold stale-
                    # but-finite psum that downstream never reads.
                    doffs = [max(0, (g + j) * 128 - c * QC) for j in range(gsz)]
                    soffs = (
                        doffs if bf_qk else [min(off, QC - 256) for off in doffs]
                    )
                    groups.append((g, gsz, doffs, soffs))
                pts = {}

                def scores(gi):
                    g, gsz, doffs, soffs = groups[gi]
                    sts = {}
                    for px in (0, 64):
                        sts[px] = ps_st.tile(
                            [128, gsz * QC], F32, tag="st", name="st"
                        )
                    for j in range(gsz):
                        kb = g + j
                        off = soffs[j]
                        for px in (0, 64):  # adjacent pair -> row-group packed
                            nc.tensor.matmul(
                                sts[px][:, j * QC + off : (j + 1) * QC],
                                lhsT=KT[hp][px : px + 64, kb * 128 : (kb + 1) * 128],
                                rhs=QT[hp][px : px + 64, c * QC + off : (c + 1) * QC],
                                start=True,
                                stop=True,
                            )
                    lead = soffs[0]  # leading fully-masked cols: skip in exp
                    for px in (0, 64):
                        pt = pt_pool.tile([128, 2 * QC], DT_PV, tag="pt", name="pt")
                        # single exp over the whole group; columns skipped by
                        # the ragged matmuls hold stale-but-finite psum, never
                        # read downstream.
                        nc.scalar.activation(
                            pt[:, lead : gsz * QC],
                            sts[px][:, lead : gsz * QC],
                            AF.Exp,
                            scale=0.125,
                        )
                        pts[(gi, px)] = pt

                def pv(gi):
                    g, gsz, doffs, _ = groups[gi]
                    for j in range(gsz):
                        kb = g + j
                        doff = kb * 128 - c * QC
                        off = doffs[j]
                        for px in (0, 64):
                            pt = pts[(gi, px)]
                            if 0 <= doff < QC:  # diagonal block: 128-wide triangle
                                blk = pt[:, j * QC + doff : j * QC + doff + 128]
                                nc.vector.tensor_mul(blk, blk, wm_t[:, 512:640])
                            h = 2 * hp + (1 if px else 0)
                            nc.tensor.matmul(
                                zps[px][0 : DH + 1, off:QC],
                                lhsT=V[kb][:, h * (DH + 1) : (h + 1) * (DH + 1)],
                                rhs=pt[:, j * QC + off : (j + 1) * QC],
                                start=(kb == 0),
                                stop=(kb == nkb - 1),
                            )

                n = len(groups)
                for gi in range(n):
                    scores(gi)
                    filler()
                    if gi >= 1:
                        pv(gi - 1)
                        filler()
                pv(n - 1)
                while fillers:
                    fillers.pop(0)()
                for px in (0, 64):
                    dstage = small.tile([128, QC], F32, tag="dstage", name="dstage")
                    nc.vector.tensor_copy(dstage[0:1, :], zps[px][DH : DH + 1, :])
                    recip = small.tile([128, QC], F32, tag="recip", name="recip")
                    nc.vector.reciprocal_approx_fast(recip, dstage)
                    bcast = small.tile([64, QC], F32, tag="bcast", name="bcast")
                    nc.gpsimd.partition_broadcast(bcast, recip[0:1, :])
                    nc.vector.tensor_mul(
                        ZT[hp][px : px + 64, c * QC : (c + 1) * QC],
                        zps[px][0:64, :],
                        bcast,
                    )

            # ---- phase B: first projections ----
            for f in qk_proj_fillers(0, qk_loads.pop(0)):
                f()
            for s in range(4):
                v_chain(s, 0, ps_st, "st")()
                v_chain(s, 1, ps_st, "st")()

            # ---- phase C1: attention c=0; qk/v projections as fillers ----
            for hp in range(NH // 2):
                if hp + 2 < NH // 2:
                    qk_loads[hp + 2] = qk_load(hp + 2)
                fillers = []
                if hp + 1 < NH // 2:
                    fillers += qk_proj_fillers(hp + 1, qk_loads.pop(hp + 1))
                if hp < 4:
                    for cch in range(NVC):
                        fillers.append(v_chain(4 + hp, cch, ps_mm, "proj"))
                if hp == 3:  # prefetch O-proj weights late in the c=0 sweep
                    for d in range(NDT):
                        t = w_pool.tile([128, DM], DT_O, tag="w", name="w")
                        nc.sync.dma_start(out=t, in_=wo[d * 128 : (d + 1) * 128, :])
                        wo_tiles.append(t)
                attn_unit(hp, 0, fillers)

            # ---- phase C2: attention c=1, o_proj (queries 0:511) interleaved ----
            # 8 o-chains spread over the 6 units (2,2,1,1,1,1) so the late
            # units keep PE filler work too
            oq = [(s, c) for s in range(4) for c in range(NVC)]
            ots = {}
            counts = [2, 2, 1, 1, 1, 1]
            for hp in range(NH // 2):
                fillers = []
                for _ in range(counts[hp]):
                    s, c = oq.pop(0)
                    if s not in ots:
                        ots[s] = out_pool.tile(
                            [128, DM], F32, tag="ostage", name="ostage"
                        )
                    fillers.append(o_chain(s, c, ots[s], ps_mm, "proj"))
                attn_unit(hp, 1, fillers)

            # ---- phase D: output projection, second half ----
            pools = [(ps_z, "z"), (ps_mm, "proj")]
            for i, s in enumerate(range(4, NQT)):
                ot = out_pool.tile([128, DM], F32, tag="ostage", name="ostage")
                for c in range(NVC):
                    pool, tag = pools[(2 * i + c) % 2]
                    o_chain(s, c, ot, pool, tag)()

    nc.compile()
    return nc


_CACHE = {}


def _get_nc(key, cfg):
    k = (key, cfg)
    if k not in _CACHE:
        _CACHE[k] = build(*key, cfg=cfg)
    return _CACHE[k]


def _prep(inputs, cfg=CFG):
    bf_qk, bf_pv, bf_o = cfg
    x = np.ascontiguousarray(np.asarray(inputs["normalized_resid_pre"], np.float32))
    dt_qk = _npdt(BF16 if bf_qk else F32R)
    dt_pv = _npdt(BF16 if bf_pv else F32R)
    dt_o = _npdt(BF16 if bf_o else F32R)
    dt_mask = _npdt(BF16 if bf_pv else F32)
    def _pack_qk(w):
        # [d_model, n_heads*d_head] -> [hp, p, d, c] slabs: row-block hp holds
        # the full d_model-contraction weights for head-pair hp, so one
        # contiguous DMA feeds all 6 lhsT tiles of a qk projection chain
        m = np.asarray(w, np.float32).transpose(1, 0, 2).reshape(DM, DM)
        m = m.reshape(NDT, 128, NH // 2, 128).transpose(2, 1, 0, 3).reshape(DM, DM)
        return np.ascontiguousarray(m).astype(dt_qk)

    wq = _pack_qk(inputs["W_Q"])
    wk = _pack_qk(inputs["W_K"])
    wv = np.ascontiguousarray(
        np.asarray(inputs["W_V"], np.float32).transpose(1, 0, 2).reshape(DM, DM)
    ).astype(dt_qk)
    wo = np.ascontiguousarray(
        np.asarray(inputs["W_O"], np.float32).reshape(DM, DM)
    ).astype(dt_o)
    bq = np.asarray(inputs["b_Q"], np.float32).reshape(NDT, 128).T
    bk = np.asarray(inputs["b_K"], np.float32).reshape(NDT, 128).T
    bv = np.asarray(inputs["b_V"], np.float32).reshape(1, DM)
    bo = np.asarray(inputs["b_O"], np.float32).reshape(1, DM)
    jj, uu = np.meshgrid(np.arange(128), np.arange(640), indexing="ij")
    wmask = (uu - 512 >= jj).astype(dt_mask)
    onesc = np.ones((128, NH), dt_pv)
    key = (
        bool(np.any(bq)),
        bool(np.any(bk)),
        bool(np.any(bv)),
        bool(np.any(bo)),
    )
    common = {
        "wq": wq, "wk": wk, "wv": wv, "wo": wo, "wmask": wmask, "onesc": onesc,
    }
    if key[0]:
        common["bq"] = np.ascontiguousarray(bq)
    if key[1]:
        common["bk"] = np.ascontiguousarray(bk)
    if key[2]:
        common["bv"] = np.ascontiguousarray(bv)
    if key[3]:
        common["bo"] = np.ascontiguousarray(bo)
    in_maps = [
        dict(common, xT=np.ascontiguousarray(x[b].T).astype(dt_qk))
        for b in range(BATCH)
    ]
    return key, in_maps


def run(inputs, trace=False, cfg=CFG, **kw):
    key, in_maps = _prep(inputs, cfg)
    nc = _get_nc(key, cfg)
    res = run_bass_kernel_spmd(
        nc, in_maps, core_ids=list(range(BATCH)), trace=trace, **kw
    )
    outs = np.stack([res.results[b]["out"] for b in range(BATCH)])
    return outs.astype(np.float32), res


def kernel(**inputs):
    out, _ = run(inputs)
    return out


if __name__ == "__main__":
    rng = np.random.default_rng(0)
    ins = {
        "normalized_resid_pre": rng.standard_normal((8, SEQ, DM)).astype(np.float32),
        "W_Q": (0.02 * rng.standard_normal((NH, DM, DH))).astype(np.float32),
        "b_Q": np.zeros((NH, DH), np.float32),
        "W_K": (0.02 * rng.standard_normal((NH, DM, DH))).astype(np.float32),
        "b_K": np.zeros((NH, DH), np.float32),
        "W_V": (0.02 * rng.standard_normal((NH, DM, DH))).astype(np.float32),
        "b_V": np.zeros((NH, DH), np.float32),
        "W_O": (0.02 * rng.standard_normal((NH, DH, DM))).astype(np.float32),
        "b_O": np.zeros((DM,), np.float32),
    }
    out = kernel(**ins)
    print("kernel output", out.shape, out.dtype, float(np.abs(out).max()))


# revision 23
# speedup vs baseline: 1.0250x; 1.0250x over previous
"""Causal multi-head attention on 8 Trainium2 NeuronCores.

Problem: nn_Attention_46643344835180
  x: [8, 1024, 768], 12 heads x 64 dh, causal softmax attention + output proj.

Sharding: data-parallel over batch (8 batch elements -> 8 cores, no collectives).

Key optimizations vs the first working version (226-265us -> ~172-179us):
  - x transposed on the HOST (input is xT, bf16): removes 48 PE
    transposes + 48 DVE casts, shortens startup.
  - All matmul inputs bf16 (halves input DMA to 6.3MB; PE rate is
    1 col/cycle either way, PSUM accumulation stays fp32). rel err 3e-3
    vs the 2e-2 gate.
  - PE kept dense so the HAM p-state never drops to 1.2GHz: attn units
    are software-pipelined (scores of group g+1 issue before PV of g)
    and independent projection chains (qk/v/o) are threaded through as
    filler closures; o_proj for the first query half is spread across
    the c=1 attention sweep.
  - DMA issue cost (~700ns/issue, serial per sequencer) amortized:
    host-packed contiguous wq/wk slabs, batched loads, issues split
    across the sync AND Activation DGE queues, transfers split into
    halves across DMA queues.
  - exp skips leading fully-masked columns; ragged f32r score matmuls
    are widened to >=256 cols when not bf16 (4 cyc/col penalty below).

Per-core dataflow (batch element b):
  xT  (DMA, host-transposed, bf16)                                    [768, 1024]
  QT = Wq_cat.T @ xT  (+bq)   heads stacked on partitions             [768, 1024]
  KT = Wk_cat.T @ xT  (+bk)                                           [768, 1024]
  V  = xT.T @ Wv_cat  (+bv)   + interleaved ones column               [1024, 12*65]
  per head h, query-chunk qc (512):
    S^T[k,q] = KT_h.T @ QT_h          keys on partitions
    P^T = exp(S^T / 8)                ScalarE, batched over 2 key-blocks
    causal: one wide-mask multiply on the partial columns
    z^T[65,512] += [V_h | 1].T @ P^T  row 64 accumulates the denominator
    ZT_h = z^T[0:64] * recip(z^T[64]) (copy -> DVE recip -> gpsimd bcast -> mul)
  out = ZT.T @ Wo_cat (+bo)                                           [1024, 768]
"""

import sys

sys.path.insert(0, "/opt/trn_rl_repo")

import ml_dtypes
import numpy as np

import concourse.bass as bass
import concourse.mybir as mybir
import concourse.tile as tile
from concourse import bacc
from concourse.bass_utils import run_bass_kernel_spmd

F32 = mybir.dt.float32
F32R = mybir.dt.float32r
BF16 = mybir.dt.bfloat16
AF = mybir.ActivationFunctionType

SEQ = 1024
DM = 768
NH = 12
DH = 64
BATCH = 8
NQT = SEQ // 128  # 8 seq tiles of 128
NDT = DM // 128  # 6 d_model tiles
QC = 512  # query chunk (moving dim)
NQC = SEQ // QC  # 2

# (bf_qk, bf_pv, bf_o): bf_qk also covers xT and wv (matmul operand dtypes
# must match: qk_proj streams xT against wq/wk, v_proj uses xT as lhsT
# against wv).
CFG = (True, True, True)


def _npdt(dt):
    return ml_dtypes.bfloat16 if dt == BF16 else np.float32


def build(with_bq, with_bk, with_bv, with_bo, cfg=CFG):
    bf_qk, bf_pv, bf_o = cfg
    DT_QK = BF16 if bf_qk else F32R  # xT, wq/wk/wv, QT/KT, scores matmul
    DT_PV = BF16 if bf_pv else F32R  # V storage, ones, P^T, PV matmul
    DT_O = BF16 if bf_o else F32R  # ZT, wo, output matmul
    DT_MASK = BF16 if bf_pv else F32

    nc = bacc.Bacc("TRN2", target_bir_lowering=False, debug=False)

    xT_d = nc.dram_tensor("xT", [DM, SEQ], DT_QK, kind="ExternalInput")
    wq = nc.dram_tensor("wq", [DM, DM], DT_QK, kind="ExternalInput")
    wk = nc.dram_tensor("wk", [DM, DM], DT_QK, kind="ExternalInput")
    wv = nc.dram_tensor("wv", [DM, DM], DT_QK, kind="ExternalInput")
    wo = nc.dram_tensor("wo", [DM, DM], DT_O, kind="ExternalInput")
    wmask = nc.dram_tensor("wmask", [128, 640], DT_MASK, kind="ExternalInput")
    onesc = nc.dram_tensor("onesc", [128, NH], DT_PV, kind="ExternalInput")
    bq = bk = bv = bo = None
    if with_bq:
        bq = nc.dram_tensor("bq", [128, NDT], F32, kind="ExternalInput")
    if with_bk:
        bk = nc.dram_tensor("bk", [128, NDT], F32, kind="ExternalInput")
    if with_bv:
        bv = nc.dram_tensor("bv", [1, DM], F32, kind="ExternalInput")
    if with_bo:
        bo = nc.dram_tensor("bo", [1, DM], F32, kind="ExternalInput")
    out = nc.dram_tensor("out", [SEQ, DM], F32, kind="ExternalOutput")

    with tile.TileContext(nc) as tc:
        with (
            tc.tile_pool(name="persist", bufs=1) as persist,
            tc.tile_pool(name="wstream", bufs=6) as w_pool,
            tc.tile_pool(name="wqk", bufs=14) as wqk_pool,
            tc.tile_pool(name="pt", bufs=6) as pt_pool,
            tc.tile_pool(name="small", bufs=3) as small,
            tc.tile_pool(name="outst", bufs=2) as out_pool,
            tc.tile_pool(name="ps_st", bufs=2, space="PSUM") as ps_st,
            tc.tile_pool(name="ps_z", bufs=2, space="PSUM") as ps_z,
            tc.tile_pool(name="ps_mm", bufs=2, space="PSUM") as ps_mm,
        ):
            # ---- constants + first-needed weights ----
            wm_t = persist.tile([128, 640], DT_MASK, tag="wmask", name="wmask")
            nc.sync.dma_start(out=wm_t, in_=wmask[:, :])

            def qk_load(hp, pieces=1):
                # wq/wk are host-packed so each head-pair's weights are one
                # contiguous [128, 768] slab. pieces>1 splits the transfer
                # across DMA queues (each queue moves only ~30-50GB/s; issue
                # costs ~700ns on the sequencer) for startup-critical loads.
                tiles = []
                for wsrc in (wq, wk):
                    t = wqk_pool.tile([128, DM], DT_QK, tag="wqk", name="wqk")
                    w = DM // pieces
                    for i in range(pieces):
                        nc.sync.dma_start(
                            out=t[:, i * w : (i + 1) * w],
                            in_=wsrc[hp * 128 : (hp + 1) * 128, i * w : (i + 1) * w],
                        )
                    tiles.append([t[:, d * 128 : (d + 1) * 128] for d in range(NDT)])
                return tiles

            # ---- input loads, split across BOTH DMA-issue queues (each
            # issue costs ~700ns serial on its sequencer): sync takes the
            # first-needed halves, the Activation queue the rest in parallel.
            xT = [
                persist.tile([128, SEQ], DT_QK, tag=f"xT{d}", name=f"xT{d}")
                for d in range(NDT)
            ]
            NVC = 2
            VC = DM // NVC  # 384
            wt = [
                w_pool.tile([128, DM], DT_QK, tag="w", name="w")
                for d in range(NDT)
            ]
            qk_loads = {0: qk_load(0, pieces=2)}
            # xT halves: c=0 halves first (what qk_proj(0)/v_proj need),
            # split between both issue queues so transfers run in parallel
            for d in range(NDT):
                eng = nc.sync if d < 3 else nc.scalar
                eng.dma_start(
                    out=xT[d][:, 0:QC], in_=xT_d[d * 128 : (d + 1) * 128, 0:QC]
                )
            for d in range(NDT):
                eng = nc.sync if d < 3 else nc.scalar
                eng.dma_start(
                    out=xT[d][:, QC:SEQ],
                    in_=xT_d[d * 128 : (d + 1) * 128, QC:SEQ],
                )
            for d in range(NDT):
                eng = nc.sync if d < 3 else nc.scalar
                eng.dma_start(out=wt[d], in_=wv[d * 128 : (d + 1) * 128, :])

            # HAM warmup: dummy matmuls on a memset scratch tile (no DMA
            # dependency) while the input DMAs land, so the first projections
            # start warm instead of at the cold 0.65-1.2GHz
            warm_in = persist.tile([128, 128], DT_QK, tag="warm0", name="warm0")
            nc.vector.memset(warm_in, 0.0)
            warm_ps = ps_mm.tile(
                [128, 128], F32, tag="proj", name="warm", padded_shape=[128, QC]
            )
            for _ in range(24):
                nc.tensor.matmul(
                    warm_ps, lhsT=warm_in, rhs=warm_in, start=True, stop=True
                )

            qk_loads[1] = qk_load(1)

            bias_tiles = {}
            if with_bq:
                t = persist.tile([128, NDT], F32, tag="bq", name="bq")
                nc.sync.dma_start(out=t, in_=bq[:, :])
                bias_tiles["bq"] = t
            if with_bk:
                t = persist.tile([128, NDT], F32, tag="bk", name="bk")
                nc.sync.dma_start(out=t, in_=bk[:, :])
                bias_tiles["bk"] = t
            if with_bv:
                t = persist.tile([128, DM], F32, tag="bv", name="bv")
                nc.sync.dma_start(out=t, in_=bv[0:1, :].to_broadcast((128, DM)))
                bias_tiles["bv"] = t
            if with_bo:
                t = persist.tile([128, DM], F32, tag="bo", name="bo")
                nc.sync.dma_start(out=t, in_=bo[0:1, :].to_broadcast((128, DM)))
                bias_tiles["bo"] = t

            # ---- persistent activations ----
            QT = [
                persist.tile([128, SEQ], DT_QK, tag=f"QT{d}", name=f"QT{d}")
                for d in range(NDT)
            ]
            KT = [
                persist.tile([128, SEQ], DT_QK, tag=f"KT{d}", name=f"KT{d}")
                for d in range(NDT)
            ]
            V = [
                persist.tile([128, NH * (DH + 1)], DT_PV, tag=f"V{s}", name=f"V{s}")
                for s in range(NQT)
            ]
            for s in range(NQT):
                vv = V[s].rearrange("p (h e) -> p h e", e=DH + 1)
                nc.scalar.dma_start(
                    out=vv[:, :, DH : DH + 1],
                    in_=onesc[:, :].rearrange("p (h o) -> p h o", o=1),
                )
            ZT = [
                persist.tile([128, SEQ], DT_O, tag=f"ZT{d}", name=f"ZT{d}")
                for d in range(NDT)
            ]

            # ---- projection chain builders (each returns a PE-work closure) ----
            def qk_chain(wts, dst, hp, c, bkey):
                def run():
                    acc = ps_mm.tile([128, QC], F32, tag="proj", name="proj")
                    for d in range(NDT):
                        nc.tensor.matmul(
                            acc,
                            lhsT=wts[d],
                            rhs=xT[d][:, c * QC : (c + 1) * QC],
                            start=(d == 0),
                            stop=(d == NDT - 1),
                        )
                    o = dst[:, c * QC : (c + 1) * QC]
                    if bkey in bias_tiles:
                        nc.vector.tensor_scalar_add(
                            o, acc, bias_tiles[bkey][:, hp : hp + 1]
                        )
                    else:
                        nc.vector.tensor_copy(o, acc)

                return run

            def qk_proj_fillers(hp, tiles):
                fs = []
                for wts, (dst, bkey) in zip(tiles, ((QT, "bq"), (KT, "bk"))):
                    for c in range(NQC):
                        fs.append(qk_chain(wts, dst[hp], hp, c, bkey))
                return fs

            def v_chain(s, c, pool, tag):
                def run():
                    acc = pool.tile(
                        [128, VC], F32, tag=tag, name="vacc",
                        padded_shape=[128, 2 * QC] if tag == "st" else [128, QC],
                    )
                    for d in range(NDT):
                        nc.tensor.matmul(
                            acc,
                            lhsT=xT[d][:, s * 128 : (s + 1) * 128],
                            rhs=wt[d][:, c * VC : (c + 1) * VC],
                            start=(d == 0),
                            stop=(d == NDT - 1),
                        )
                    nh2 = VC // DH  # heads per chunk (6)
                    o = V[s].rearrange("p (h e) -> p h e", e=DH + 1)[
                        :, c * nh2 : (c + 1) * nh2, 0:DH
                    ]
                    if "bv" in bias_tiles:
                        nc.vector.tensor_add(
                            o,
                            acc.rearrange("p (h e) -> p h e", e=DH),
                            bias_tiles["bv"][:, c * VC : (c + 1) * VC].rearrange(
                                "p (h e) -> p h e", e=DH
                            ),
                        )
                    else:
                        nc.vector.tensor_copy(
                            o, acc.rearrange("p (h e) -> p h e", e=DH)
                        )

                return run

            wo_tiles = []

            def o_chain(s, c, ot, pool, tag):
                def run():
                    acc = pool.tile(
                        [128, VC], F32, tag=tag, name="oacc", padded_shape=[128, QC]
                    )
                    for d in range(NDT):
                        nc.tensor.matmul(
                            acc,
                            lhsT=ZT[d][:, s * 128 : (s + 1) * 128],
                            rhs=wo_tiles[d][:, c * VC : (c + 1) * VC],
                            start=(d == 0),
                            stop=(d == NDT - 1),
                        )
                    o = ot[:, c * VC : (c + 1) * VC]
                    if "bo" in bias_tiles:
                        nc.vector.tensor_add(
                            o, acc, bias_tiles["bo"][:, c * VC : (c + 1) * VC]
                        )
                    else:
                        nc.vector.tensor_copy(o, acc)
                    if c == NVC - 1:
                        nc.sync.dma_start(out=out[s * 128 : (s + 1) * 128, :], in_=ot)

                return run

            # ---- pipelined attention unit (head-pair hp, query chunk c) ----
            def attn_unit(hp, c, fillers=()):
                fillers = list(fillers)

                def filler():
                    # keep the last filler in reserve: it runs after the final
                    # PV group, covering the denominator-chain latency tail
                    if len(fillers) > 1:
                        fillers.pop(0)()

                zps = {}
                for px in (0, 64):  # head A in partitions 0:64, B in 64:128
                    zps[px] = ps_z.tile([128, QC], F32, tag="z", name="z")
                nkb = 4 * (c + 1)  # causal: key blocks 0..nkb-1
                groups = []
                for g in range(0, nkb, 2):
                    gsz = min(2, nkb - g)
                    # columns [0:doff) of a diagonal block are fully causal-
                    # masked: skip them in scores and PV (ragged-N). For f32r
                    # scores, widen to >=256 cols (narrow f32r matmuls run at
                    # 4 cycles/col at full clock); extra columns hold stale-
                    # but-finite psum that downstream never reads.
                    doffs = [max(0, (g + j) * 128 - c * QC) for j in range(gsz)]
                    soffs = (
                        doffs if bf_qk else [min(off, QC - 256) for off in doffs]
                    )
                    groups.append((g, gsz, doffs, soffs))
                pts = {}

                def scores(gi):
                    g, gsz, doffs, soffs = groups[gi]
                    sts = {}
                    for px in (0, 64):
                        sts[px] = ps_st.tile(
                            [128, gsz * QC], F32, tag="st", name="st"
                        )
                    for j in range(gsz):
                        kb = g + j
                        off = soffs[j]
                        for px in (0, 64):  # adjacent pair -> row-group packed
                            nc.tensor.matmul(
                                sts[px][:, j * QC + off : (j + 1) * QC],
                                lhsT=KT[hp][px : px + 64, kb * 128 : (kb + 1) * 128],
                                rhs=QT[hp][px : px + 64, c * QC + off : (c + 1) * QC],
                                start=True,
                                stop=True,
                            )
                    lead = soffs[0]  # leading fully-masked cols: skip in exp
                    for px in (0, 64):
                        pt = pt_pool.tile([128, 2 * QC], DT_PV, tag="pt", name="pt")
                        # single exp over the whole group; columns skipped by
                        # the ragged matmuls hold stale-but-finite psum, never
                        # read downstream.
                        nc.scalar.activation(
                            pt[:, lead : gsz * QC],
                            sts[px][:, lead : gsz * QC],
                            AF.Exp,
                            scale=0.125,
                        )
                        pts[(gi, px)] = pt

                def pv(gi):
                    g, gsz, doffs, _ = groups[gi]
                    for j in range(gsz):
                        kb = g + j
                        doff = kb * 128 - c * QC
                        off = doffs[j]
                        for px in (0, 64):
                            pt = pts[(gi, px)]
                            if 0 <= doff < QC:  # diagonal block: 128-wide triangle
                                blk = pt[:, j * QC + doff : j * QC + doff + 128]
                                nc.vector.tensor_mul(blk, blk, wm_t[:, 512:640])
                            h = 2 * hp + (1 if px else 0)
                            nc.tensor.matmul(
                                zps[px][0 : DH + 1, off:QC],
                                lhsT=V[kb][:, h * (DH + 1) : (h + 1) * (DH + 1)],
                                rhs=pt[:, j * QC + off : (j + 1) * QC],
                                start=(kb == 0),
                                stop=(kb == nkb - 1),
                            )

                n = len(groups)
                for gi in range(n):
                    scores(gi)
                    filler()
                    if gi >= 1:
                        pv(gi - 1)
                        filler()
                pv(n - 1)
                while fillers:
                    fillers.pop(0)()
                for px in (0, 64):
                    dstage = small.tile([128, QC], F32, tag="dstage", name="dstage")
                    nc.vector.tensor_copy(dstage[0:1, :], zps[px][DH : DH + 1, :])
                    recip = small.tile([128, QC], F32, tag="recip", name="recip")
                    nc.vector.reciprocal_approx_fast(recip, dstage)
                    bcast = small.tile([64, QC], F32, tag="bcast", name="bcast")
                    nc.gpsimd.partition_broadcast(bcast, recip[0:1, :])
                    nc.vector.tensor_mul(
                        ZT[hp][px : px + 64, c * QC : (c + 1) * QC],
                        zps[px][0:64, :],
                        bcast,
                    )

            # ---- phase B: first projections ----
            for f in qk_proj_fillers(0, qk_loads.pop(0)):
                f()
            for s in range(4):
                v_chain(s, 0, ps_st, "st")()
                v_chain(s, 1, ps_st, "st")()

            # ---- phase C1: attention c=0; qk/v projections as fillers ----
            for hp in range(NH // 2):
                if hp + 2 < NH // 2:
                    qk_loads[hp + 2] = qk_load(hp + 2)
                fillers = []
                if hp + 1 < NH // 2:
                    fillers += qk_proj_fillers(hp + 1, qk_loads.pop(hp + 1))
                if hp < 4:
                    for cch in range(NVC):
                        fillers.append(v_chain(4 + hp, cch, ps_mm, "proj"))
                if hp == 3:  # prefetch O-proj weights late in the c=0 sweep
                    for d in range(NDT):
                        t = w_pool.tile([128, DM], DT_O, tag="w", name="w")
                        nc.sync.dma_start(out=t, in_=wo[d * 128 : (d + 1) * 128, :])
                        wo_tiles.append(t)
                attn_unit(hp, 0, fillers)

            # ---- phase C2: attention c=1, o_proj (queries 0:511) interleaved ----
            # 8 o-chains spread over the 6 units (2,2,1,1,1,1) so the late
            # units keep PE filler work too
            oq = [(s, c) for s in range(4) for c in range(NVC)]
            ots = {}
            counts = [2, 2, 1, 1, 1, 1]
            for hp in range(NH // 2):
                fillers = []
                for _ in range(counts[hp]):
                    s, c = oq.pop(0)
                    if s not in ots:
                        ots[s] = out_pool.tile(
                            [128, DM], F32, tag="ostage", name="ostage"
                        )
                    fillers.append(o_chain(s, c, ots[s], ps_mm, "proj"))
                attn_unit(hp, 1, fillers)

            # ---- phase D: output projection, second half ----
            pools = [(ps_z, "z"), (ps_mm, "proj")]
            chains = []
            for i, s in enumerate(range(4, NQT)):
                ot = out_pool.tile([128, DM], F32, tag="ostage", name="ostage")
                for c in range(NVC):
                    pool, tag = pools[(2 * i + c) % 2]
                    chains.append((s, c, ot, pool, tag))
            # first four chains: emit d0..d4 for all, THEN the d5 matmuls.
            # d5 waits on the last attention unit's ZT writes (behind its
            # denominator chain); 20 independent matmuls hide that latency.
            accs = {}
            for s, c, ot, pool, tag in chains[:4]:
                acc = pool.tile(
                    [128, VC], F32, tag=tag, name="oacc", padded_shape=[128, QC]
                )
                for d in range(NDT - 1):
                    nc.tensor.matmul(
                        acc,
                        lhsT=ZT[d][:, s * 128 : (s + 1) * 128],
                        rhs=wo_tiles[d][:, c * VC : (c + 1) * VC],
                        start=(d == 0),
                        stop=False,
                    )
                accs[(s, c)] = acc
            for s, c, ot, pool, tag in chains[:4]:
                acc = accs[(s, c)]
                nc.tensor.matmul(
                    acc,
                    lhsT=ZT[NDT - 1][:, s * 128 : (s + 1) * 128],
                    rhs=wo_tiles[NDT - 1][:, c * VC : (c + 1) * VC],
                    start=False,
                    stop=True,
                )
                o = ot[:, c * VC : (c + 1) * VC]
                if "bo" in bias_tiles:
                    nc.vector.tensor_add(
                        o, acc, bias_tiles["bo"][:, c * VC : (c + 1) * VC]
                    )
                else:
                    nc.vector.tensor_copy(o, acc)
                if c == NVC - 1:
                    nc.sync.dma_start(out=out[s * 128 : (s + 1) * 128, :], in_=ot)
            for s, c, ot, pool, tag in chains[4:]:
                o_chain(s, c, ot, pool, tag)()

    nc.compile()
    return nc


_CACHE = {}


def _get_nc(key, cfg):
    k = (key, cfg)
    if k not in _CACHE:
        _CACHE[k] = build(*key, cfg=cfg)
    return _CACHE[k]


def _prep(inputs, cfg=CFG):
    bf_qk, bf_pv, bf_o = cfg
    x = np.ascontiguousarray(np.asarray(inputs["normalized_resid_pre"], np.float32))
    dt_qk = _npdt(BF16 if bf_qk else F32R)
    dt_pv = _npdt(BF16 if bf_pv else F32R)
    dt_o = _npdt(BF16 if bf_o else F32R)
    dt_mask = _npdt(BF16 if bf_pv else F32)
    def _pack_qk(w):
        # [d_model, n_heads*d_head] -> [hp, p, d, c] slabs: row-block hp holds
        # the full d_model-contraction weights for head-pair hp, so one
        # contiguous DMA feeds all 6 lhsT tiles of a qk projection chain
        m = np.asarray(w, np.float32).transpose(1, 0, 2).reshape(DM, DM)
        m = m.reshape(NDT, 128, NH // 2, 128).transpose(2, 1, 0, 3).reshape(DM, DM)
        return np.ascontiguousarray(m).astype(dt_qk)

    wq = _pack_qk(inputs["W_Q"])
    wk = _pack_qk(inputs["W_K"])
    wv = np.ascontiguousarray(
        np.asarray(inputs["W_V"], np.float32).transpose(1, 0, 2).reshape(DM, DM)
    ).astype(dt_qk)
    wo = np.ascontiguousarray(
        np.asarray(inputs["W_O"], np.float32).reshape(DM, DM)
    ).astype(dt_o)
    bq = np.asarray(inputs["b_Q"], np.float32).reshape(NDT, 128).T
    bk = np.asarray(inputs["b_K"], np.float32).reshape(NDT, 128).T
    bv = np.asarray(inputs["b_V"], np.float32).reshape(1, DM)
    bo = np.asarray(inputs["b_O"], np.float32).reshape(1, DM)
    jj, uu = np.meshgrid(np.arange(128), np.arange(640), indexing="ij")
    wmask = (uu - 512 >= jj).astype(dt_mask)
    onesc = np.ones((128, NH), dt_pv)
    key = (
        bool(np.any(bq)),
        bool(np.any(bk)),
        bool(np.any(bv)),
        bool(np.any(bo)),
    )
    common = {
        "wq": wq, "wk": wk, "wv": wv, "wo": wo, "wmask": wmask, "onesc": onesc,
    }
    if key[0]:
        common["bq"] = np.ascontiguousarray(bq)
    if key[1]:
        common["bk"] = np.ascontiguousarray(bk)
    if key[2]:
        common["bv"] = np.ascontiguousarray(bv)
    if key[3]:
        common["bo"] = np.ascontiguousarray(bo)
    in_maps = [
        dict(common, xT=np.ascontiguousarray(x[b].T).astype(dt_qk))
        for b in range(BATCH)
    ]
    return key, in_maps


def run(inputs, trace=False, cfg=CFG, **kw):
    key, in_maps = _prep(inputs, cfg)
    nc = _get_nc(key, cfg)
    res = run_bass_kernel_spmd(
        nc, in_maps, core_ids=list(range(BATCH)), trace=trace, **kw
    )
    outs = np.stack([res.results[b]["out"] for b in range(BATCH)])
    return outs.astype(np.float32), res


def kernel(**inputs):
    out, _ = run(inputs)
    return out


if __name__ == "__main__":
    rng = np.random.default_rng(0)
    ins = {
        "normalized_resid_pre": rng.standard_normal((8, SEQ, DM)).astype(np.float32),
        "W_Q": (0.02 * rng.standard_normal((NH, DM, DH))).astype(np.float32),
        "b_Q": np.zeros((NH, DH), np.float32),
        "W_K": (0.02 * rng.standard_normal((NH, DM, DH))).astype(np.float32),
        "b_K": np.zeros((NH, DH), np.float32),
        "W_V": (0.02 * rng.standard_normal((NH, DM, DH))).astype(np.float32),
        "b_V": np.zeros((NH, DH), np.float32),
        "W_O": (0.02 * rng.standard_normal((NH, DH, DM))).astype(np.float32),
        "b_O": np.zeros((DM,), np.float32),
    }
    out = kernel(**ins)
    print("kernel output", out.shape, out.dtype, float(np.abs(out).max()))
